# revision 1
# baseline (speedup 1.0000x reference)
"""BlipAttention kernel for 8 Trainium2 NeuronCores.

Strategy: data-parallel over batch (16 batches -> 2 per core), no collectives.
Per core: fused QKV projection + 16-head scaled-dot-product attention + output
projection on the PE, bf16 matmuls with fp32 PSUM accumulation.

Layout tricks:
  - x is transposed on-chip (PE transpose) to feature-major x^T so the
    contraction dim (D) lives on SBUF partitions for all projections.
  - q,k are projected with full 128-wide M tiles (feature-packed), then
    redistributed to per-head [88, S] tiles with SBUF->SBUF DMAs (DMA can
    shift partition offsets; compute engines cannot).
  - scores are computed TRANSPOSED (k-tokens on partitions) so softmax
    denominators come for free from the PV matmul: v is stored token-major
    with ones-columns appended per head (97-wide groups, cols 88..96 = 1.0),
    which makes the PV matmul emit  sum_k exp(scores)  at PSUM partition 96
    (a legal quadrant offset for the subsequent reciprocal read).
  - 1/denom is broadcast across partitions with a rank-1 (K=1) matmul.
  - attention outputs are DMA-packed back to 128-wide K tiles so the output
    projection contracts with K=128 pieces.
  - biases are applied via ACT bias (feature-major q,k) or rank-1 ones
    matmuls (token-major v / output projection).
  - weights are host-converted to bf16 and DMA'd in batched stripes on the
    otherwise-idle GpSimd DMA queue.
"""

import contextlib

import numpy as np
import ml_dtypes

import concourse.bass as bass
import concourse.tile as tile
from concourse import bacc, mybir
from concourse.bass_utils import run_bass_kernel_spmd

F32 = mybir.dt.float32
F32R = mybir.dt.float32r
BF16 = mybir.dt.bfloat16

N_CORES = 8
B_TOTAL, S, D = 16, 577, 1408
H, HD = 16, 88
SCALE = HD ** -0.5
B = B_TOTAL // N_CORES          # batches per core = 2
T = B * S                       # tokens per core = 1154
SP = S + 1                      # padded per-batch token span = 578
KT = D // 128                   # 11 k-tiles over D
MT = 2 * KT                     # 22 m-tiles over the packed q|k blocks
TT = (S + 127) // 128           # 5 token tiles per batch (128,128,128,128,65)
VG = 97                         # v group width per head: 88 v cols + 9 ones
DEN = 96                        # psum partition of the softmax denominator

# moving-dim chunks for 578-wide spans: (logical col, width)
CH_S = [(0, 512), (512, 66)]
# chunks for 1408-wide spans
CH_D = [(0, 512), (512, 512), (1024, 384)]


def _tok_tiles():
    out = []
    for tt in range(TT):
        t0 = tt * 128
        out.append((tt, t0, min(128, S - t0)))
    return out


def build_program():
    nc = bacc.Bacc("TRN2", target_bir_lowering=False, debug=False,
                   num_devices=N_CORES)

    x_ap = nc.dram_tensor("x", [T, D], F32, kind="ExternalInput").ap()
    wqkv_ap = nc.dram_tensor("w_qkv_bf", [D, 3 * D], BF16, kind="ExternalInput").ap()
    bq_col_ap = nc.dram_tensor("b_qkv_col", [2 * D, 1], F32, kind="ExternalInput").ap()
    bv_row_ap = nc.dram_tensor("b_v_row", [1, D], BF16, kind="ExternalInput").ap()
    wp_ap = nc.dram_tensor("w_proj_bf", [D, D], BF16, kind="ExternalInput").ap()
    bp_row_ap = nc.dram_tensor("b_proj_row", [1, D], BF16, kind="ExternalInput").ap()
    ones_ap = nc.dram_tensor("ones_f", [1, 128], F32, kind="ExternalInput").ap()
    ones_bf_ap = nc.dram_tensor("ones_bf", [128, 128], BF16, kind="ExternalInput").ap()
    ident_ap = nc.dram_tensor("ident_f", [128, 128], F32, kind="ExternalInput").ap()
    vones_ap = nc.dram_tensor("vones_bf", [128, H * VG], BF16, kind="ExternalInput").ap()
    out_ap = nc.dram_tensor("out", [T, D], F32, kind="ExternalOutput").ap()


    with tile.TileContext(nc) as tc, contextlib.ExitStack() as ctx:
        p_xraw = ctx.enter_context(tc.tile_pool(name="xraw", bufs=2))
        p_xT = ctx.enter_context(tc.tile_pool(name="xT", bufs=11))
        p_vsb = ctx.enter_context(tc.tile_pool(name="vsb", bufs=5))
        p_qksb = ctx.enter_context(tc.tile_pool(name="qksb", bufs=5))
        p_qk = ctx.enter_context(tc.tile_pool(name="qk", bufs=34))
        p_expT = ctx.enter_context(tc.tile_pool(name="expT", bufs=10))
        p_pvsb = ctx.enter_context(tc.tile_pool(name="pvsb", bufs=4))
        p_attn = ctx.enter_context(tc.tile_pool(name="attn", bufs=6))
        p_apk = ctx.enter_context(tc.tile_pool(name="apk", bufs=12))
        p_rec = ctx.enter_context(tc.tile_pool(name="rec", bufs=4))
        p_wq = ctx.enter_context(tc.tile_pool(name="wq", bufs=6))
        p_wv = ctx.enter_context(tc.tile_pool(name="wv", bufs=12))
        p_wp = ctx.enter_context(tc.tile_pool(name="wp", bufs=13))
        p_bias = ctx.enter_context(tc.tile_pool(name="bias", bufs=8))
        p_brow = ctx.enter_context(tc.tile_pool(name="brow", bufs=2))
        p_const = ctx.enter_context(tc.tile_pool(name="const", bufs=1))
        p_ost = ctx.enter_context(tc.tile_pool(name="ost", bufs=2))

        psum = ctx.enter_context(tc.tile_pool(name="psum", bufs=8, space="PSUM"))

        def ps():
            return psum.tile([128, 512], F32, tag="ps", name="pst")

        ident = p_const.tile([128, 128], F32, tag="ident")
        nc.sync.dma_start(ident[:], ident_ap[:])
        ones = p_const.tile([1, 128], F32R, tag="ones")
        nc.sync.dma_start(ones[:], ones_ap[0:1, :].bitcast(F32R))
        ones_bf = p_const.tile([1, 128], BF16, tag="ones_bf")
        nc.sync.dma_start(ones_bf[:], ones_bf_ap[0:1, :])

        bvr = p_brow.tile([1, D], BF16, tag="bvr")
        nc.sync.dma_start(bvr[:], bv_row_ap[:])
        bpr = p_brow.tile([1, D], BF16, tag="bpr")
        nc.sync.dma_start(bpr[:], bp_row_ap[:])

        # x^T tiles cover both batches; allocated once, written per batch.
        xT = [p_xT.tile([128, B * SP], BF16, tag="xT", name=f"xT{k}")
              for k in range(KT)]

        # v tiles are allocated once: per-batch v-projection rewrites only the
        # 88 v-columns of each 97-group; the ones-columns are written once.
        vsb = [p_vsb.tile([128, H * VG], BF16, tag="vsb", name=f"vsb{tt}")
               for tt in range(TT)]
        for tt in range(TT):
            nc.sync.dma_start(vsb[tt][:], vones_ap[:])

        for b in range(B):
            boff = b * SP

            # ---- stage A: load x (token-major) and transpose to x^T ----
            for tt, t0, ts in _tok_tiles():
                xr = p_xraw.tile([128, D], F32, tag="xraw")
                nc.sync.dma_start(xr[0:ts, :], x_ap[b * S + t0: b * S + t0 + ts, :])
                for k in range(KT):
                    pt = ps()
                    nc.tensor.transpose(pt[0:128, 0:ts], xr[0:ts, k * 128:(k + 1) * 128],
                                        ident[0:ts, 0:ts])
                    nc.vector.tensor_copy(xT[k][:, boff + t0: boff + t0 + ts],
                                          pt[0:128, 0:ts])
            # fill padded token column (keeps downstream values finite)
            for k in range(KT):
                nc.sync.dma_start(xT[k][:, boff + S: boff + S + 1],
                                  ones_bf_ap[:, 0:1])

            # ---- stage B: v projection, token-major, head-interleaved ----
            for (c0, w) in CH_D:
                wvs = []
                for k in range(KT):
                    wv = p_wv.tile([128, 512], BF16, tag="wv")
                    nc.gpsimd.dma_start(
                        wv[:, 0:w],
                        wqkv_ap[k * 128:(k + 1) * 128, 2 * D + c0: 2 * D + c0 + w])
                    wvs.append(wv)
                for tt, t0, ts in _tok_tiles():
                    pv = ps()
                    for k in range(KT):
                        nc.tensor.matmul(pv[0:ts, 0:w],
                                         xT[k][:, boff + t0: boff + t0 + ts],
                                         wvs[k][:, 0:w], start=(k == 0), stop=False)
                    nc.tensor.matmul(pv[0:ts, 0:w], ones_bf[:, 0:ts],
                                     bvr[:, c0:c0 + w], start=False, stop=True)
                    # split per head into the 97-wide groups
                    h0 = c0 // HD
                    h1 = min(H - 1, (c0 + w - 1) // HD)
                    for h in range(h0, h1 + 1):
                        s0 = max(c0, h * HD)
                        s1 = min(c0 + w, (h + 1) * HD)
                        if s1 <= s0:
                            continue
                        nc.vector.tensor_copy(
                            vsb[tt][0:ts, h * VG + (s0 - h * HD): h * VG + (s1 - h * HD)],
                            pv[0:ts, s0 - c0: s1 - c0])

            # ---- stage C1: packed q|k projection (M=128 tiles) + head
            # redistribution via partition-shifting SBUF->SBUF DMAs ----
            qh = [None] * H
            kh = [None] * H
            frag = {}   # head tile -> next partition row to fill
            for m in range(MT):
                col = m * 128
                wq = p_wq.tile([128, KT * 128], BF16, tag="wq")
                nc.gpsimd.dma_start(
                    wq[:].rearrange("p (k c) -> p k c", k=KT),
                    wqkv_ap[:, col: col + 128].rearrange("(k p) c -> p k c", p=128))
                pts = []
                for (lc, w) in CH_S:
                    pt = ps()
                    for k in range(KT):
                        nc.tensor.matmul(pt[0:128, 0:w],
                                         wq[:, k * 128:(k + 1) * 128],
                                         xT[k][:, boff + lc: boff + lc + w],
                                         start=(k == 0), stop=(k == KT - 1))
                    pts.append(pt)
                bq = p_bias.tile([128, 1], F32, tag="bias")
                nc.sync.dma_start(bq[:], bq_col_ap[col: col + 128, :])
                qksb = p_qksb.tile([128, SP], BF16, tag="qksb")
                for (lc, w), pt in zip(CH_S, pts):
                    nc.scalar.activation(qksb[:, lc:lc + w], pt[0:128, 0:w],
                                         mybir.ActivationFunctionType.Identity,
                                         bias=bq[:])
                # ship finished head rows out of this m-tile
                which, dst = (0, qh) if m < KT else (1, kh)
                f_lo, f_hi = (m - which * KT) * 128, (m - which * KT) * 128 + 128
                for h in range(f_lo // HD, min(H, (f_hi + HD - 1) // HD)):
                    s0 = max(f_lo, h * HD)
                    s1 = min(f_hi, (h + 1) * HD)
                    if s1 <= s0:
                        continue
                    if dst[h] is None:
                        dst[h] = p_qk.tile([HD, SP], BF16, tag="qk",
                                           name=f"qk_{b}_{which}_{h}")
                    r0 = s0 - h * HD
                    nc.sync.dma_start(dst[h][r0: r0 + (s1 - s0), :],
                                      qksb[s0 - f_lo: s1 - f_lo, :])

            # ---- stage C2: per-head attention ----
            apk = [p_apk.tile([128, SP], BF16, tag="apk", name=f"apk_{b}_{k}")
                   for k in range(KT)]

            def finish_norm(h, rec, pvs):
                # broadcast 1/denom over partitions via rank-1 matmul, then
                # normalize and pack into 128-wide K tiles for the projection
                at = p_attn.tile([HD, SP], BF16, tag="attn", name=f"at_{b}_{h}")
                for (lc, w) in CH_S:
                    pb = ps()
                    nc.tensor.matmul(pb[0:HD, 0:w], ones[:, 0:HD],
                                     rec[:, lc:lc + w], start=True, stop=True)
                    nc.vector.tensor_mul(at[:, lc:lc + w], pvs[:, lc:lc + w],
                                         pb[0:HD, 0:w])
                f0 = h * HD
                k0, r0 = f0 // 128, f0 % 128
                n0 = min(HD, 128 - r0)
                nc.sync.dma_start(apk[k0][r0: r0 + n0, :], at[0:n0, :])
                if n0 < HD:
                    nc.sync.dma_start(apk[k0 + 1][0: HD - n0, :], at[n0:HD, :])

            pending = None
            for h in range(H):
                # transposed scores + exp, per k-token tile
                expT = []
                for tt, t0, ts in _tok_tiles():
                    pts = []
                    for (lc, w) in CH_S:
                        pt = ps()
                        nc.tensor.matmul(pt[0:ts, 0:w],
                                         kh[h][:, t0:t0 + ts], qh[h][:, lc:lc + w],
                                         start=True, stop=True)
                        pts.append(pt)
                    et = p_expT.tile([128, SP], BF16, tag="expT")
                    expT.append(et)
                    for (lc, w), pt in zip(CH_S, pts):
                        nc.scalar.activation(et[0:ts, lc:lc + w], pt[0:ts, 0:w],
                                             mybir.ActivationFunctionType.Exp,
                                             scale=SCALE)

                # PV with fused denominator at psum partition 96
                pvs_ps = []
                for (lc, w) in CH_S:
                    pv = ps()
                    for tt, t0, ts in _tok_tiles():
                        nc.tensor.matmul(pv[0:VG, 0:w],
                                         vsb[tt][0:ts, h * VG:(h + 1) * VG],
                                         expT[tt][0:ts, lc:lc + w],
                                         start=(tt == 0), stop=(tt == TT - 1))
                    pvs_ps.append(pv)

                rec = p_rec.tile([1, SP], F32R, tag="rec", name=f"rec_{b}_{h}")
                with nc.allow_low_precision(reason="softmax reciprocal"):
                    for (lc, w), pv in zip(CH_S, pvs_ps):
                        nc.vector.reciprocal(rec[:, lc:lc + w],
                                             pv[DEN:DEN + 1, 0:w])
                pvs = p_pvsb.tile([HD, SP], F32, tag="pvsb", name=f"pvs_{b}_{h}")
                for (lc, w), pv in zip(CH_S, pvs_ps):
                    nc.scalar.activation(pvs[:, lc:lc + w], pv[0:HD, 0:w],
                                         mybir.ActivationFunctionType.Copy)
                # deferred by one head so the rank-1 broadcast never stalls
                # the in-order PE queue waiting on the DVE reciprocal
                if pending is not None:
                    finish_norm(*pending)
                pending = (h, rec, pvs)
            finish_norm(*pending)

            # ---- stage D: output projection (token-major, K=128 pieces) ----
            for (c0, w) in CH_D:
                wps = []
                for k in range(KT):
                    wpt = p_wp.tile([128, 512], BF16, tag="wp")
                    nc.gpsimd.dma_start(wpt[:, 0:w],
                                        wp_ap[k * 128:(k + 1) * 128, c0:c0 + w])
                    wps.append(wpt)
                for tt, t0, ts in _tok_tiles():
                    po = ps()
                    for k in range(KT):
                        nc.tensor.matmul(po[0:ts, 0:w], apk[k][:, t0:t0 + ts],
                                         wps[k][:, 0:w], start=(k == 0), stop=False)
                    nc.tensor.matmul(po[0:ts, 0:w], ones_bf[:, 0:ts],
                                     bpr[:, c0:c0 + w], start=False, stop=True)
                    ot = p_ost.tile([128, 512], F32, tag="ost")
                    nc.scalar.activation(ot[0:ts, 0:w], po[0:ts, 0:w],
                                         mybir.ActivationFunctionType.Copy)
                    nc.sync.dma_start(
                        out_ap[b * S + t0: b * S + t0 + ts, c0:c0 + w], ot[0:ts, 0:w])

    nc.compile()
    return nc


_NC_CACHE = None


def _get_nc():
    global _NC_CACHE
    if _NC_CACHE is None:
        _NC_CACHE = build_program()
    return _NC_CACHE


def make_in_maps(hidden_states, w_qkv, b_qkv, w_proj, b_proj):
    hidden_states = np.asarray(hidden_states, dtype=np.float32)
    w_qkv = np.ascontiguousarray(np.asarray(w_qkv, dtype=np.float32))
    b_qkv = np.asarray(b_qkv, dtype=np.float32)
    w_proj = np.asarray(w_proj, dtype=np.float32)
    b_proj = np.asarray(b_proj, dtype=np.float32)

    wqkv_bf = w_qkv.astype(ml_dtypes.bfloat16)
    wp_bf = w_proj.astype(ml_dtypes.bfloat16)
    bq_col = b_qkv[: 2 * D].reshape(2 * D, 1).copy()
    bv_row = b_qkv[2 * D:].astype(ml_dtypes.bfloat16).reshape(1, D).copy()
    bp_row = b_proj.astype(ml_dtypes.bfloat16).reshape(1, D).copy()
    ones_f = np.ones((1, 128), np.float32)
    ones_bf = np.ones((128, 128), ml_dtypes.bfloat16)
    ident_f = np.eye(128, dtype=np.float32)
    vones_bf = np.ones((128, H * VG), ml_dtypes.bfloat16)

    in_maps = []
    for c in range(N_CORES):
        xs = hidden_states[c * B:(c + 1) * B].reshape(T, D)
        in_maps.append({
            "x": np.ascontiguousarray(xs),
            "w_qkv_bf": wqkv_bf,
            "b_qkv_col": bq_col,
            "b_v_row": bv_row,
            "w_proj_bf": wp_bf,
            "b_proj_row": bp_row,
            "ones_f": ones_f,
            "ones_bf": ones_bf,
            "ident_f": ident_f,
            "vones_bf": vones_bf,
        })
    return in_maps


def kernel(hidden_states, w_qkv, b_qkv, w_proj, b_proj):
    nc = _get_nc()
    in_maps = make_in_maps(hidden_states, w_qkv, b_qkv, w_proj, b_proj)
    res = run_bass_kernel_spmd(nc, in_maps, list(range(N_CORES)))
    out = np.concatenate(
        [res.results[c]["out"].reshape(B, S, D) for c in range(N_CORES)], axis=0)
    return out.astype(np.float32)


if __name__ == "__main__":
    rng = np.random.default_rng(0)
    hs = rng.standard_normal((B_TOTAL, S, D), dtype=np.float32)
    wq = rng.standard_normal((D, 3 * D), dtype=np.float32) * D ** -0.5
    bq = rng.standard_normal(3 * D).astype(np.float32) * 0.02
    wp = rng.standard_normal((D, D), dtype=np.float32) * D ** -0.5
    bp = rng.standard_normal(D).astype(np.float32) * 0.02
    o = kernel(hidden_states=hs, w_qkv=wq, b_qkv=bq, w_proj=wp, b_proj=bp)
    print(o.shape, o.dtype)



# revision 4
# speedup vs baseline: 1.2382x; 1.2382x over previous
"""BlipAttention kernel for 8 Trainium2 NeuronCores (v2).

Data-parallel over batch (16 batches -> 2 per core), no collectives.

v2 strategy (vs v1): keep the PE dense end-to-end so the HAM clock gate
never re-throttles (v1 ran ~half the kernel at 1.2 GHz), and make the
scalar engine do nothing but softmax exp.

  - x is transposed to feature-major x^T ON THE HOST and uploaded bf16
    (stage-A PE transposes and the fp32 x upload are gone).
  - 4-phase software pipeline, interleaved at EMISSION level (the Tile
    scheduler's per-engine ready-heaps pop in emission order):
      P1: v-proj (both batches) + q|k-proj(b0), coarsely interleaved
      P2: attention(b0) with q|k-proj(b1) matmuls as PE filler between
          the softmax dependency stalls
      P3: attention(b1) with out-proj(b0) as PE filler
      P4: out-proj(b1)
  - scores/PV/C1 psum tiles span TWO PSUM banks ([128,1024] f32) so each
    (head, k-tile) needs ONE exp activation over the full 578-token span
    (ACT per-instruction overhead is 352 cycles -- halving the count
    saves ~45us), and chunk matmuls share each LDWEIGHTS load.
  - every PSUM->SBUF drain is on the vector engine (tensor_scalar_add /
    tensor_copy), leaving ACT 100% for exp.
  - weights are host-prepacked into contiguous per-stripe layouts so all
    weight DMAs are large and contiguous.
  - softmax denominators come free from ones-columns in the v tiles
    (PV emits sum_k exp at psum partition 96), reciprocal on DVE,
    broadcast across partitions with a rank-1 (K=1) matmul.
"""

import contextlib
from collections import deque

import numpy as np
import ml_dtypes

import concourse.bass as bass
import concourse.tile as tile
from concourse import bacc, mybir
from concourse.bass_utils import run_bass_kernel_spmd

F32 = mybir.dt.float32
F32R = mybir.dt.float32r
BF16 = mybir.dt.bfloat16
AF = mybir.ActivationFunctionType

N_CORES = 8
B_TOTAL, S, D = 16, 577, 1408
H, HD = 16, 88
SCALE = HD ** -0.5
B = B_TOTAL // N_CORES          # batches per core = 2
T = B * S                       # tokens per core = 1154
SP = S + 1                      # padded per-batch token span = 578
KT = D // 128                   # 11 k-tiles over D
MT = 2 * KT                     # 22 m-tiles over the packed q|k features
TT = (S + 127) // 128           # 5 token tiles per batch
VG = 97                         # v group width per head: 88 v cols + 9 ones
DEN = 96                        # psum partition of the softmax denominator

TOK = [(tt, tt * 128, min(128, S - tt * 128)) for tt in range(TT)]
CH_D = [(0, 512), (512, 512), (1024, 384)]    # chunks over 1408 v-features
DCG = [(0, 1024), (1024, 384)]                # out-proj column groups


class Filler:
    """Queue of emission generators; take(n) emits ~n PE-cycles of filler."""

    def __init__(self):
        self.q = deque()
        self.credit = 0

    def add(self, gen):
        # prime: first yield emits the piece's DMA prefetches only
        try:
            next(gen)
            self.q.append(gen)
        except StopIteration:
            pass

    def take(self, n):
        self.credit += n
        while self.credit > 0 and self.q:
            try:
                self.credit -= next(self.q[0])
            except StopIteration:
                self.q.popleft()

    def drain(self):
        while self.q:
            try:
                next(self.q[0])
            except StopIteration:
                self.q.popleft()


def build_program():
    nc = bacc.Bacc("TRN2", target_bir_lowering=False, debug=False,
                   num_devices=N_CORES)

    xT_ap = nc.dram_tensor("xT_bf", [D, B * SP], BF16, kind="ExternalInput").ap()
    wqkm_ap = nc.dram_tensor("wqk_m", [128, MT * KT * 128], BF16,
                             kind="ExternalInput").ap()
    wv_ap = nc.dram_tensor("wv_r", [128, KT * D], BF16, kind="ExternalInput").ap()
    wp_ap = nc.dram_tensor("wp_r", [128, KT * D], BF16, kind="ExternalInput").ap()
    bqk_ap = nc.dram_tensor("b_qk_col", [2 * D, 1], F32, kind="ExternalInput").ap()
    bv_ap = nc.dram_tensor("b_v_row", [1, D], BF16, kind="ExternalInput").ap()
    bp_ap = nc.dram_tensor("b_p_row", [1, D], BF16, kind="ExternalInput").ap()
    ones_f_ap = nc.dram_tensor("ones_f", [1, 128], F32, kind="ExternalInput").ap()
    ones_bf_ap = nc.dram_tensor("ones_bf", [1, 128], BF16, kind="ExternalInput").ap()
    out_ap = nc.dram_tensor("out", [T, D], F32, kind="ExternalOutput").ap()

    with tile.TileContext(nc) as tc, contextlib.ExitStack() as ctx:
        p_xT = ctx.enter_context(tc.tile_pool(name="xT", bufs=KT))
        p_w = ctx.enter_context(tc.tile_pool(name="w", bufs=KT))
        p_wqk = ctx.enter_context(tc.tile_pool(name="wqk", bufs=3))
        p_vsb = ctx.enter_context(tc.tile_pool(name="vsb", bufs=2 * TT))
        p_qk = ctx.enter_context(tc.tile_pool(name="qk", bufs=2 * H))
        p_qksb = ctx.enter_context(tc.tile_pool(name="qksb", bufs=2))
        p_expT = ctx.enter_context(tc.tile_pool(name="expT", bufs=6))
        p_pvs = ctx.enter_context(tc.tile_pool(name="pvs", bufs=2))
        p_at = ctx.enter_context(tc.tile_pool(name="at", bufs=2))
        p_apk = ctx.enter_context(tc.tile_pool(name="apk", bufs=2 * KT))
        p_ost = ctx.enter_context(tc.tile_pool(name="ost", bufs=2))
        p_rec = ctx.enter_context(tc.tile_pool(name="rec", bufs=2))
        p_bias = ctx.enter_context(tc.tile_pool(name="bias", bufs=4))
        p_brow = ctx.enter_context(tc.tile_pool(name="brow", bufs=2))
        p_const = ctx.enter_context(tc.tile_pool(name="const", bufs=1))

        # PSUM: 8 banks total = 1x2 (sc) + 2x2 (pv) + 1x2 (big)
        p_sc = ctx.enter_context(tc.tile_pool(name="psc", bufs=1, space="PSUM"))
        p_pv = ctx.enter_context(tc.tile_pool(name="ppv", bufs=2, space="PSUM"))
        p_big = ctx.enter_context(tc.tile_pool(name="pbig", bufs=1, space="PSUM"))

        # ---- constants ----
        ones_f = p_const.tile([1, 128], F32R, tag="ones_f")
        nc.sync.dma_start(ones_f[:], ones_f_ap[0:1, :].bitcast(F32R))
        ones_bf = p_const.tile([1, 128], BF16, tag="ones_bf")
        nc.sync.dma_start(ones_bf[:], ones_bf_ap[0:1, :])
        bvr = p_brow.tile([1, D], BF16, tag="brow", name="bvr")
        nc.sync.dma_start(bvr[:], bv_ap[:])
        bpr = p_brow.tile([1, D], BF16, tag="brow", name="bpr")
        nc.sync.dma_start(bpr[:], bp_ap[:])

        # ---- x^T tiles (both batches), host-transposed bf16 ----
        xT = []
        for k in range(KT):
            t = p_xT.tile([128, B * SP], BF16, tag="xT", name=f"xT{k}")
            nc.sync.dma_start(t[:], xT_ap[k * 128:(k + 1) * 128, :])
            xT.append(t)

        # ---- resident v-proj weights (11 tiles; wp reuses the slots) ----
        wv = []
        for k in range(KT):
            t = p_w.tile([128, D], BF16, tag="w", name=f"wv{k}")
            nc.gpsimd.dma_start(t[:], wv_ap[:, k * D:(k + 1) * D])
            wv.append(t)

        # ---- v tiles: ones-filled via gpsimd memset (denominator cols) ----
        vsb = {}
        for b in range(B):
            vsb[b] = []
            for tt in range(TT):
                t = p_vsb.tile([128, H * VG], BF16, tag="vsb",
                               name=f"vsb_{b}_{tt}")
                nc.gpsimd.memset(t[:], 1.0)
                vsb[b].append(t)

        qkt = {}

        def qk_tile(b, which, h):
            key = (b, which, h)
            if key not in qkt:
                qkt[key] = p_qk.tile([HD, SP], BF16, tag="qk",
                                     name=f"qk_{b}_{which}_{h}")
            return qkt[key]

        apk = {b: [p_apk.tile([128, SP], BF16, tag="apk", name=f"apk_{b}_{k}")
                   for k in range(KT)] for b in range(B)}

        # ================= piece generators =================

        def gen_b_chunk(b, ci):
            """v projection for batch b, feature chunk ci (token-major)."""
            c0, w = CH_D[ci]
            boff = b * SP
            yield 0
            sA = p_sc.tile([128, 1024], F32, tag="ps", name=f"bps_{b}_{ci}_a")
            sB = p_pv.tile([128, 1024], F32, tag="ps", name=f"bps_{b}_{ci}_b")
            sC = p_pv.tile([128, 1024], F32, tag="ps", name=f"bps_{b}_{ci}_c")
            slot = [(sA, 0), (sA, 512), (sB, 0), (sB, 512), (sC, 0)]
            for k in range(KT):
                for tt, t0, ts in TOK:
                    tl, off = slot[tt]
                    nc.tensor.matmul(tl[0:ts, off:off + w],
                                     xT[k][:, boff + t0:boff + t0 + ts],
                                     wv[k][:, c0:c0 + w],
                                     start=(k == 0), stop=False)
                yield TT * w
            for tt, t0, ts in TOK:
                tl, off = slot[tt]
                nc.tensor.matmul(tl[0:ts, off:off + w], ones_bf[:, 0:ts],
                                 bvr[:, c0:c0 + w], start=False, stop=True)
            yield TT * w
            h0, h1 = c0 // HD, min(H - 1, (c0 + w - 1) // HD)
            for tt, t0, ts in TOK:
                tl, off = slot[tt]
                for h in range(h0, h1 + 1):
                    s0, s1 = max(c0, h * HD), min(c0 + w, (h + 1) * HD)
                    if s1 <= s0:
                        continue
                    nc.vector.tensor_copy(
                        vsb[b][tt][0:ts, h * VG + (s0 - h * HD):
                                   h * VG + (s1 - h * HD)],
                        tl[0:ts, off + (s0 - c0):off + (s1 - c0)])
                yield 0

        def gen_c1_m(b, m):
            """q|k projection m-tile (feature-major) + head redistribution."""
            wq = p_wqk.tile([128, KT * 128], BF16, tag="wqk", name=f"wq_{b}_{m}")
            nc.gpsimd.dma_start(wq[:], wqkm_ap[:, m * 1408:(m + 1) * 1408])
            bqt = p_bias.tile([128, 1], F32, tag="bias", name=f"bq_{b}_{m}")
            nc.sync.dma_start(bqt[:], bqk_ap[m * 128:(m + 1) * 128, :])
            yield 0
            boff = b * SP
            pt = p_big.tile([128, 1024], F32, tag="ps", name=f"c1p_{b}_{m}")
            for k in range(KT):
                nc.tensor.matmul(pt[0:128, 0:512], wq[:, k * 128:(k + 1) * 128],
                                 xT[k][:, boff:boff + 512],
                                 start=(k == 0), stop=(k == KT - 1))
                nc.tensor.matmul(pt[0:128, 512:578], wq[:, k * 128:(k + 1) * 128],
                                 xT[k][:, boff + 512:boff + 578],
                                 start=(k == 0), stop=(k == KT - 1))
                yield SP
            qksb = p_qksb.tile([128, SP], BF16, tag="qksb", name=f"qs_{b}_{m}")
            nc.vector.tensor_scalar_add(qksb[:, 0:SP], pt[0:128, 0:SP], bqt[:])
            which = 0 if m < KT else 1
            f_lo = (m - which * KT) * 128
            f_hi = f_lo + 128
            for h in range(f_lo // HD, min(H, (f_hi + HD - 1) // HD)):
                s0, s1 = max(f_lo, h * HD), min(f_hi, (h + 1) * HD)
                if s1 <= s0:
                    continue
                t = qk_tile(b, which, h)
                r0 = s0 - h * HD
                nc.sync.dma_start(t[r0:r0 + (s1 - s0), :],
                                  qksb[s0 - f_lo:s1 - f_lo, :])
            yield 0

        wp = []

        def emit_wp_load():
            k = len(wp)
            if k < KT:
                t = p_w.tile([128, D], BF16, tag="w", name=f"wp{k}")
                nc.gpsimd.dma_start(t[:], wp_ap[:, k * D:(k + 1) * D])
                wp.append(t)

        def gen_d_piece(b, tt, t0, ts, cgi, pool):
            """out-proj for batch b, token tile tt, column group cgi."""
            c0, w = DCG[cgi]
            yield 0
            po = pool.tile([128, 1024], F32, tag="ps", name=f"po_{b}_{tt}_{cgi}")
            for k in range(KT):
                if w == 1024:
                    nc.tensor.matmul(po[0:ts, 0:512], apk[b][k][:, t0:t0 + ts],
                                     wp[k][:, c0:c0 + 512],
                                     start=(k == 0), stop=False)
                    nc.tensor.matmul(po[0:ts, 512:1024], apk[b][k][:, t0:t0 + ts],
                                     wp[k][:, c0 + 512:c0 + 1024],
                                     start=(k == 0), stop=False)
                else:
                    nc.tensor.matmul(po[0:ts, 0:w], apk[b][k][:, t0:t0 + ts],
                                     wp[k][:, c0:c0 + w],
                                     start=(k == 0), stop=False)
                yield w
            if w == 1024:
                nc.tensor.matmul(po[0:ts, 0:512], ones_bf[:, 0:ts],
                                 bpr[:, c0:c0 + 512], start=False, stop=True)
                nc.tensor.matmul(po[0:ts, 512:1024], ones_bf[:, 0:ts],
                                 bpr[:, c0 + 512:c0 + 1024], start=False, stop=True)
            else:
                nc.tensor.matmul(po[0:ts, 0:w], ones_bf[:, 0:ts],
                                 bpr[:, c0:c0 + w], start=False, stop=True)
            ot = p_ost.tile([128, 1024], F32, tag="ost", name=f"ot_{b}_{tt}_{cgi}")
            nc.vector.tensor_copy(ot[0:ts, 0:w], po[0:ts, 0:w])
            nc.sync.dma_start(out_ap[b * S + t0:b * S + t0 + ts, c0:c0 + w],
                              ot[0:ts, 0:w])
            yield 0

        # ================= attention =================

        def finish_head(b, h, rec, pvs):
            pb = p_pv.tile([128, 1024], F32, tag="ps", name=f"pb_{b}_{h}")
            nc.tensor.matmul(pb[0:HD, 0:512], ones_f[:, 0:HD], rec[:, 0:512],
                             start=True, stop=True)
            nc.tensor.matmul(pb[0:HD, 512:578], ones_f[:, 0:HD], rec[:, 512:578],
                             start=True, stop=True)
            at = p_at.tile([HD, SP], BF16, tag="at", name=f"at_{b}_{h}")
            nc.vector.tensor_mul(at[:, 0:SP], pvs[:, 0:SP], pb[0:HD, 0:SP])
            f0 = h * HD
            k0, r0 = f0 // 128, f0 % 128
            n0 = min(HD, 128 - r0)
            nc.sync.dma_start(apk[b][k0][r0:r0 + n0, :], at[0:n0, :])
            if n0 < HD:
                nc.sync.dma_start(apk[b][k0 + 1][0:HD - n0, :], at[n0:HD, :])

        def attention_phase(b, filler, fill_exp, fill_pv, gate=None):
            pend = None
            for h in range(H):
                if gate is not None:
                    gate(h)
                qh_t = qk_tile(b, 0, h)
                kh_t = qk_tile(b, 1, h)
                expTs = []
                for tt, t0, ts in TOK:
                    pt = p_sc.tile([128, 1024], F32, tag="ps",
                                   name=f"sc_{b}_{h}_{tt}")
                    nc.tensor.matmul(pt[0:ts, 0:512], kh_t[:, t0:t0 + ts],
                                     qh_t[:, 0:512], start=True, stop=True)
                    nc.tensor.matmul(pt[0:ts, 512:578], kh_t[:, t0:t0 + ts],
                                     qh_t[:, 512:578], start=True, stop=True)
                    et = p_expT.tile([128, SP], BF16, tag="expT",
                                     name=f"et_{b}_{h}_{tt}")
                    nc.scalar.activation(et[0:ts, 0:SP], pt[0:ts, 0:SP],
                                         AF.Exp, scale=SCALE)
                    expTs.append(et)
                    filler.take(fill_exp)
                pv = p_pv.tile([128, 1024], F32, tag="ps", name=f"pv_{b}_{h}")
                for tt, t0, ts in TOK:
                    et = expTs[tt]
                    vsl = vsb[b][tt][0:ts, h * VG:(h + 1) * VG]
                    nc.tensor.matmul(pv[0:VG, 0:512], vsl, et[0:ts, 0:512],
                                     start=(tt == 0), stop=(tt == TT - 1))
                    nc.tensor.matmul(pv[0:VG, 512:578], vsl, et[0:ts, 512:578],
                                     start=(tt == 0), stop=(tt == TT - 1))
                    filler.take(fill_pv)
                rec = p_rec.tile([1, SP], F32R, tag="rec", name=f"rec_{b}_{h}")
                with nc.allow_low_precision(reason="softmax reciprocal"):
                    nc.vector.reciprocal(rec[:, 0:SP], pv[DEN:DEN + 1, 0:SP])
                pvs = p_pvs.tile([HD, SP], F32, tag="pvs", name=f"pvs_{b}_{h}")
                nc.vector.tensor_copy(pvs[:, 0:SP], pv[0:HD, 0:SP])
                if pend is not None:
                    finish_head(b, *pend)
                pend = (h, rec, pvs)
            finish_head(b, *pend)

        # ================= phase drivers =================

        def drain(gen):
            for _ in gen:
                pass

        # ---- P1: v-proj(b0,b1) + q|k-proj(b0), coarse interleave ----
        c1b0 = [gen_c1_m(0, m) for m in range(MT)]
        p1 = []
        bpieces = [(b, ci) for b in range(B) for ci in range(3)]
        mi = 0
        for i, (b, ci) in enumerate(bpieces):
            p1.append(gen_b_chunk(b, ci))
            take = 4 if i < 4 else 3
            for _ in range(take):
                if mi < MT:
                    p1.append(c1b0[mi])
                    mi += 1
        while mi < MT:
            p1.append(c1b0[mi])
            mi += 1
        primed = 0
        for i in range(len(p1)):
            while primed < min(i + 3, len(p1)):
                try:
                    next(p1[primed])
                except StopIteration:
                    pass
                primed += 1
            drain(p1[i])

        # ---- P2: attention(b0) with q|k-proj(b1) as filler ----
        f2 = Filler()
        c1b1 = [gen_c1_m(1, m) for m in range(MT)]
        gstate = {"j": 0}

        def gate2(h):
            jm = min((88 * (h + 1)) // 128, KT - 1)
            while gstate["j"] <= jm:
                j = gstate["j"]
                f2.add(c1b1[j])          # q m-tile j
                f2.add(c1b1[KT + j])     # k m-tile j
                emit_wp_load()
                gstate["j"] += 1

        attention_phase(0, f2, fill_exp=1100, fill_pv=650, gate=gate2)
        f2.drain()
        while len(wp) < KT:
            emit_wp_load()

        # ---- P3: attention(b1) with out-proj(b0) as filler ----
        f3 = Filler()
        for tt, t0, ts in TOK:
            for cgi in range(2):
                f3.add(gen_d_piece(0, tt, t0, ts, cgi, p_big))
        attention_phase(1, f3, fill_exp=650, fill_pv=390)
        f3.drain()

        # ---- P4: out-proj(b1), psum rotating across all three pools ----
        pools4 = [p_big, p_sc, p_pv]
        pi = 0
        d4 = []
        for tt, t0, ts in TOK:
            for cgi in range(2):
                d4.append(gen_d_piece(1, tt, t0, ts, cgi, pools4[pi % 3]))
                pi += 1
        primed = 0
        for i in range(len(d4)):
            while primed < min(i + 2, len(d4)):
                try:
                    next(d4[primed])
                except StopIteration:
                    pass
                primed += 1
            drain(d4[i])

    nc.compile()
    return nc


_NC_CACHE = None


def _get_nc():
    global _NC_CACHE
    if _NC_CACHE is None:
        _NC_CACHE = build_program()
    return _NC_CACHE


def make_in_maps(hidden_states, w_qkv, b_qkv, w_proj, b_proj):
    bf16 = ml_dtypes.bfloat16
    hs = np.asarray(hidden_states, dtype=np.float32)
    w_qkv = np.ascontiguousarray(np.asarray(w_qkv, dtype=np.float32))
    b_qkv = np.asarray(b_qkv, dtype=np.float32)
    w_proj = np.ascontiguousarray(np.asarray(w_proj, dtype=np.float32))
    b_proj = np.asarray(b_proj, dtype=np.float32)

    # q|k weights -> m-stripe layout: wqk_m[p, m*1408 + k*128 + c]
    #   = w_qkv[k*128 + p, m*128 + c]
    wqk = w_qkv[:, :2 * D].reshape(KT, 128, MT, 128)
    wqk_m = np.ascontiguousarray(
        wqk.transpose(1, 2, 0, 3).reshape(128, MT * KT * 128)).astype(bf16)
    # v / proj weights -> k-stripe layout: w_r[p, k*1408 + c] = w[k*128+p, c]
    wv_r = np.ascontiguousarray(
        w_qkv[:, 2 * D:].reshape(KT, 128, D).transpose(1, 0, 2)
        .reshape(128, KT * D)).astype(bf16)
    wp_r = np.ascontiguousarray(
        w_proj.reshape(KT, 128, D).transpose(1, 0, 2)
        .reshape(128, KT * D)).astype(bf16)

    bqk_col = np.ascontiguousarray(b_qkv[:2 * D].reshape(2 * D, 1))
    bv_row = np.ascontiguousarray(b_qkv[2 * D:].reshape(1, D)).astype(bf16)
    bp_row = np.ascontiguousarray(b_proj.reshape(1, D)).astype(bf16)
    ones_f = np.ones((1, 128), np.float32)
    ones_bf = np.ones((1, 128), bf16)

    in_maps = []
    for c in range(N_CORES):
        xt = np.ones((D, B * SP), bf16)
        for b in range(B):
            xs = hs[c * B + b]                       # [S, D]
            xt[:, b * SP:b * SP + S] = xs.T.astype(bf16)
        in_maps.append({
            "xT_bf": xt,
            "wqk_m": wqk_m,
            "wv_r": wv_r,
            "wp_r": wp_r,
            "b_qk_col": bqk_col,
            "b_v_row": bv_row,
            "b_p_row": bp_row,
            "ones_f": ones_f,
            "ones_bf": ones_bf,
        })
    return in_maps


def kernel(hidden_states, w_qkv, b_qkv, w_proj, b_proj):
    nc = _get_nc()
    in_maps = make_in_maps(hidden_states, w_qkv, b_qkv, w_proj, b_proj)
    res = run_bass_kernel_spmd(nc, in_maps, list(range(N_CORES)))
    out = np.concatenate(
        [res.results[c]["out"].reshape(B, S, D) for c in range(N_CORES)], axis=0)
    return out.astype(np.float32)


if __name__ == "__main__":
    rng = np.random.default_rng(0)
    hs = rng.standard_normal((B_TOTAL, S, D), dtype=np.float32)
    wq = rng.standard_normal((D, 3 * D), dtype=np.float32) * D ** -0.5
    bq = rng.standard_normal(3 * D).astype(np.float32) * 0.02
    wp = rng.standard_normal((D, D), dtype=np.float32) * D ** -0.5
    bp = rng.standard_normal(D).astype(np.float32) * 0.02
    o = kernel(hidden_states=hs, w_qkv=wq, b_qkv=bq, w_proj=wp, b_proj=bp)
    print(o.shape, o.dtype)


# revision 14
# speedup vs baseline: 1.5698x; 1.2678x over previous
"""BlipAttention kernel for 8 Trainium2 NeuronCores (v2).

Data-parallel over batch (16 batches -> 2 per core), no collectives.

v2 strategy (vs v1): keep the PE dense end-to-end so the HAM clock gate
never re-throttles (v1 ran ~half the kernel at 1.2 GHz), and make the
scalar engine do nothing but softmax exp.

  - x is transposed to feature-major x^T ON THE HOST and uploaded bf16
    (stage-A PE transposes and the fp32 x upload are gone).
  - 4-phase software pipeline, interleaved at EMISSION level (the Tile
    scheduler's per-engine ready-heaps pop in emission order):
      P1: v-proj (both batches) + q|k-proj(b0), coarsely interleaved
      P2: attention(b0) with q|k-proj(b1) matmuls as PE filler between
          the softmax dependency stalls
      P3: attention(b1) with out-proj(b0) as PE filler
      P4: out-proj(b1)
  - scores/PV/C1 psum tiles span TWO PSUM banks ([128,1024] f32) so each
    (head, k-tile) needs ONE exp activation over the full 578-token span
    (ACT per-instruction overhead is 352 cycles -- halving the count
    saves ~45us), and chunk matmuls share each LDWEIGHTS load.
  - every PSUM->SBUF drain is on the vector engine (tensor_scalar_add /
    tensor_copy), leaving ACT 100% for exp.
  - weights are host-prepacked into contiguous per-stripe layouts so all
    weight DMAs are large and contiguous.
  - softmax denominators come free from ones-columns in the v tiles
    (PV emits sum_k exp at psum partition 96), reciprocal on DVE,
    broadcast across partitions with a rank-1 (K=1) matmul.
"""

import contextlib
from collections import deque

import numpy as np
import ml_dtypes

import concourse.bass as bass
import concourse.tile as tile
from concourse import bacc, mybir
from concourse.bass_utils import run_bass_kernel_spmd

F32 = mybir.dt.float32
F32R = mybir.dt.float32r
BF16 = mybir.dt.bfloat16
AF = mybir.ActivationFunctionType

N_CORES = 8
B_TOTAL, S, D = 16, 577, 1408
H, HD = 16, 88
SCALE = HD ** -0.5
B = B_TOTAL // N_CORES          # batches per core = 2
T = B * S                       # tokens per core = 1154
SP = S + 1                      # padded per-batch token span = 578
KT = D // 128                   # 11 k-tiles over D
MT = 2 * KT                     # 22 m-tiles over the packed q|k features
TT = (S + 127) // 128           # 5 token tiles per batch
VG = 97                         # v group width per head: 88 v cols + 9 ones
DEN = 96                        # psum partition of the softmax denominator

TOK = [(tt, tt * 128, min(128, S - tt * 128)) for tt in range(TT)]
CH_D = [(0, 512), (512, 512), (1024, 384)]    # chunks over 1408 v-features
DCG = [(0, 1024), (1024, 384)]                # out-proj column groups


class Filler:
    """Queue of emission generators; take(n) emits ~n PE-cycles of filler."""

    def __init__(self):
        self.q = deque()
        self.credit = 0

    def add(self, gen):
        # prime: first yield emits the piece's DMA prefetches only
        try:
            next(gen)
            self.q.append(gen)
        except StopIteration:
            pass

    def take(self, n):
        self.credit += n
        while self.credit > 0 and self.q:
            try:
                self.credit -= next(self.q[0])
            except StopIteration:
                self.q.popleft()

    def drain(self):
        while self.q:
            try:
                next(self.q[0])
            except StopIteration:
                self.q.popleft()


def build_program():
    nc = bacc.Bacc("TRN2", target_bir_lowering=False, debug=False,
                   num_devices=N_CORES)

    xT_ap = nc.dram_tensor("xT_bf", [D, B * SP], BF16, kind="ExternalInput").ap()
    wqkm_ap = nc.dram_tensor("wqk_m", [128, MT * KT * 128], BF16,
                             kind="ExternalInput").ap()
    wv_ap = nc.dram_tensor("wv_r", [128, KT * D], BF16, kind="ExternalInput").ap()
    wp_ap = nc.dram_tensor("wp_r", [128, KT * D], BF16, kind="ExternalInput").ap()
    bqk_ap = nc.dram_tensor("b_qk_col", [2 * D, 1], F32, kind="ExternalInput").ap()
    bv_ap = nc.dram_tensor("b_v_row", [1, D], BF16, kind="ExternalInput").ap()
    bp_ap = nc.dram_tensor("b_p_row", [1, D], BF16, kind="ExternalInput").ap()
    ones_f_ap = nc.dram_tensor("ones_f", [1, 128], F32, kind="ExternalInput").ap()
    ones_bf_ap = nc.dram_tensor("ones_bf", [1, 128], BF16, kind="ExternalInput").ap()
    out_ap = nc.dram_tensor("out", [T, D], F32, kind="ExternalOutput").ap()

    with tile.TileContext(nc) as tc, contextlib.ExitStack() as ctx:
        p_xT = ctx.enter_context(tc.tile_pool(name="xT", bufs=KT))
        p_w = ctx.enter_context(tc.tile_pool(name="w", bufs=KT))
        p_wqk = ctx.enter_context(tc.tile_pool(name="wqk", bufs=3))
        p_vsb = ctx.enter_context(tc.tile_pool(name="vsb", bufs=2 * TT))
        p_qk = ctx.enter_context(tc.tile_pool(name="qk", bufs=2 * H))
        p_qksb = ctx.enter_context(tc.tile_pool(name="qksb", bufs=2))
        p_expT = ctx.enter_context(tc.tile_pool(name="expT", bufs=6))
        p_pvs = ctx.enter_context(tc.tile_pool(name="pvs", bufs=2))
        p_at = ctx.enter_context(tc.tile_pool(name="at", bufs=2))
        p_apk = ctx.enter_context(tc.tile_pool(name="apk", bufs=2 * KT))
        p_ost = ctx.enter_context(tc.tile_pool(name="ost", bufs=2))
        p_rec = ctx.enter_context(tc.tile_pool(name="rec", bufs=2))
        p_bias = ctx.enter_context(tc.tile_pool(name="bias", bufs=4))
        p_brow = ctx.enter_context(tc.tile_pool(name="brow", bufs=2))
        p_const = ctx.enter_context(tc.tile_pool(name="const", bufs=1))

        # PSUM: 8 banks total = 1x2 (sc) + 2x2 (pv) + 1x2 (big)
        p_sc = ctx.enter_context(tc.tile_pool(name="psc", bufs=1, space="PSUM"))
        p_pv = ctx.enter_context(tc.tile_pool(name="ppv", bufs=2, space="PSUM"))
        p_big = ctx.enter_context(tc.tile_pool(name="pbig", bufs=1, space="PSUM"))

        # ---- constants ----
        ones_f = p_const.tile([1, 128], F32R, tag="ones_f")
        nc.sync.dma_start(ones_f[:], ones_f_ap[0:1, :].bitcast(F32R))
        ones_bf = p_const.tile([1, 128], BF16, tag="ones_bf")
        nc.sync.dma_start(ones_bf[:], ones_bf_ap[0:1, :])
        bvr = p_brow.tile([1, D], BF16, tag="brow", name="bvr")
        nc.sync.dma_start(bvr[:], bv_ap[:])
        bpr = p_brow.tile([1, D], BF16, tag="brow", name="bpr")
        nc.sync.dma_start(bpr[:], bp_ap[:])

        # ---- x^T tiles (both batches), host-transposed bf16 ----
        xT = []
        for k in range(KT):
            t = p_xT.tile([128, B * SP], BF16, tag="xT", name=f"xT{k}")
            eng = nc.sync if k % 2 == 0 else nc.scalar
            eng.dma_start(t[:], xT_ap[k * 128:(k + 1) * 128, :])
            xT.append(t)

        # ---- resident v-proj weights (11 tiles; wp reuses the slots) ----
        wv = []
        for k in range(KT):
            t = p_w.tile([128, D], BF16, tag="w", name=f"wv{k}")
            nc.gpsimd.dma_start(t[:], wv_ap[:, k * D:(k + 1) * D])
            wv.append(t)

        # ---- v tiles: ones-filled via gpsimd memset (denominator cols) ----
        vsb = {}
        for b in range(B):
            vsb[b] = []
            for tt in range(TT):
                t = p_vsb.tile([128, H * VG], BF16, tag="vsb",
                               name=f"vsb_{b}_{tt}")
                nc.vector.memset(t[:], 1.0)
                vsb[b].append(t)

        qkt = {}

        def qk_tile(b, which, h):
            key = (b, which, h)
            if key not in qkt:
                qkt[key] = p_qk.tile([HD, SP], BF16, tag="qk",
                                     name=f"qk_{b}_{which}_{h}")
            return qkt[key]

        apk = {b: [p_apk.tile([128, SP], BF16, tag="apk", name=f"apk_{b}_{k}")
                   for k in range(KT)] for b in range(B)}

        # ================= piece generators =================

        def gen_b_chunk(b, ci):
            """v projection for batch b, feature chunk ci (token-major)."""
            c0, w = CH_D[ci]
            boff = b * SP
            yield 0
            sA = p_sc.tile([128, 1024], F32, tag="ps", name=f"bps_{b}_{ci}_a")
            sB = p_pv.tile([128, 1024], F32, tag="ps", name=f"bps_{b}_{ci}_b")
            sC = p_pv.tile([128, 1024], F32, tag="ps", name=f"bps_{b}_{ci}_c")
            slot = [(sA, 0), (sA, 512), (sB, 0), (sB, 512), (sC, 0)]
            for k in range(KT):
                for tt, t0, ts in TOK:
                    tl, off = slot[tt]
                    nc.tensor.matmul(tl[0:ts, off:off + w],
                                     xT[k][:, boff + t0:boff + t0 + ts],
                                     wv[k][:, c0:c0 + w],
                                     start=(k == 0), stop=False)
                yield TT * w
            for tt, t0, ts in TOK:
                tl, off = slot[tt]
                nc.tensor.matmul(tl[0:ts, off:off + w], ones_bf[:, 0:ts],
                                 bvr[:, c0:c0 + w], start=False, stop=True)
            yield TT * w
            h0, h1 = c0 // HD, min(H - 1, (c0 + w - 1) // HD)
            for tt, t0, ts in TOK:
                tl, off = slot[tt]
                for h in range(h0, h1 + 1):
                    s0, s1 = max(c0, h * HD), min(c0 + w, (h + 1) * HD)
                    if s1 <= s0:
                        continue
                    nc.vector.tensor_copy(
                        vsb[b][tt][0:ts, h * VG + (s0 - h * HD):
                                   h * VG + (s1 - h * HD)],
                        tl[0:ts, off + (s0 - c0):off + (s1 - c0)])
                yield 0

        def gen_c1_m(b, m):
            """q|k projection m-tile (feature-major) + head redistribution."""
            wq = p_wqk.tile([128, KT * 128], BF16, tag="wqk", name=f"wq_{b}_{m}")
            nc.gpsimd.dma_start(wq[:], wqkm_ap[:, m * 1408:(m + 1) * 1408])
            bqt = p_bias.tile([128, 1], F32, tag="bias", name=f"bq_{b}_{m}")
            nc.sync.dma_start(bqt[:], bqk_ap[m * 128:(m + 1) * 128, :])
            yield 0
            boff = b * SP
            pt = p_big.tile([128, 1024], F32, tag="ps", name=f"c1p_{b}_{m}")
            for k in range(KT):
                nc.tensor.matmul(pt[0:128, 0:512], wq[:, k * 128:(k + 1) * 128],
                                 xT[k][:, boff:boff + 512],
                                 start=(k == 0), stop=(k == KT - 1))
                nc.tensor.matmul(pt[0:128, 512:578], wq[:, k * 128:(k + 1) * 128],
                                 xT[k][:, boff + 512:boff + 578],
                                 start=(k == 0), stop=(k == KT - 1))
                yield SP
            qksb = p_qksb.tile([128, SP], BF16, tag="qksb", name=f"qs_{b}_{m}")
            nc.vector.tensor_scalar_add(qksb[:, 0:SP], pt[0:128, 0:SP], bqt[:])
            which = 0 if m < KT else 1
            f_lo = (m - which * KT) * 128
            f_hi = f_lo + 128
            for h in range(f_lo // HD, min(H, (f_hi + HD - 1) // HD)):
                s0, s1 = max(f_lo, h * HD), min(f_hi, (h + 1) * HD)
                if s1 <= s0:
                    continue
                t = qk_tile(b, which, h)
                r0 = s0 - h * HD
                nc.sync.dma_start(t[r0:r0 + (s1 - s0), :],
                                  qksb[s0 - f_lo:s1 - f_lo, :])
            yield 0

        wp = []

        def emit_wp_load():
            k = len(wp)
            if k < KT:
                t = p_w.tile([128, D], BF16, tag="w", name=f"wp{k}")
                nc.gpsimd.dma_start(t[:], wp_ap[:, k * D:(k + 1) * D])
                wp.append(t)

        def gen_d_piece(b, tt, t0, ts, cgi, pool):
            """out-proj for batch b, token tile tt, column group cgi."""
            c0, w = DCG[cgi]
            yield 0
            po = pool.tile([128, 1024], F32, tag="ps", name=f"po_{b}_{tt}_{cgi}")
            for k in range(KT):
                if w == 1024:
                    nc.tensor.matmul(po[0:ts, 0:512], apk[b][k][:, t0:t0 + ts],
                                     wp[k][:, c0:c0 + 512],
                                     start=(k == 0), stop=False)
                    nc.tensor.matmul(po[0:ts, 512:1024], apk[b][k][:, t0:t0 + ts],
                                     wp[k][:, c0 + 512:c0 + 1024],
                                     start=(k == 0), stop=False)
                else:
                    nc.tensor.matmul(po[0:ts, 0:w], apk[b][k][:, t0:t0 + ts],
                                     wp[k][:, c0:c0 + w],
                                     start=(k == 0), stop=False)
                yield w
            if w == 1024:
                nc.tensor.matmul(po[0:ts, 0:512], ones_bf[:, 0:ts],
                                 bpr[:, c0:c0 + 512], start=False, stop=True)
                nc.tensor.matmul(po[0:ts, 512:1024], ones_bf[:, 0:ts],
                                 bpr[:, c0 + 512:c0 + 1024], start=False, stop=True)
            else:
                nc.tensor.matmul(po[0:ts, 0:w], ones_bf[:, 0:ts],
                                 bpr[:, c0:c0 + w], start=False, stop=True)
            ot = p_ost.tile([128, 1024], F32, tag="ost", name=f"ot_{b}_{tt}_{cgi}")
            nc.vector.tensor_copy(ot[0:ts, 0:w], po[0:ts, 0:w])
            nc.sync.dma_start(out_ap[b * S + t0:b * S + t0 + ts, c0:c0 + w],
                              ot[0:ts, 0:w])
            yield 0

        # ================= attention =================

        def finish_head(b, h, rec, pvs):
            pb = p_pv.tile([128, 1024], F32, tag="ps", name=f"pb_{b}_{h}")
            nc.tensor.matmul(pb[0:HD, 0:512], ones_bf[:, 0:HD],
                             rec[:, 0:512], start=True, stop=True)
            nc.tensor.matmul(pb[0:HD, 512:578], ones_bf[:, 0:HD],
                             rec[:, 512:578], start=True, stop=True)
            at = p_at.tile([HD, SP], BF16, tag="at", name=f"at_{b}_{h}")
            nc.vector.tensor_mul(at[:, 0:SP], pvs[0:HD, 0:SP], pb[0:HD, 0:SP])
            f0 = h * HD
            k0, r0 = f0 // 128, f0 % 128
            n0 = min(HD, 128 - r0)
            nc.sync.dma_start(apk[b][k0][r0:r0 + n0, :], at[0:n0, :])
            if n0 < HD:
                nc.sync.dma_start(apk[b][k0 + 1][0:HD - n0, :], at[n0:HD, :])

        def attention_phase(b, filler, fill_exp, fill_pv, gate=None):
            pend = None
            for h in range(H):
                if gate is not None:
                    gate(h)
                qh_t = qk_tile(b, 0, h)
                kh_t = qk_tile(b, 1, h)
                expTs = []
                for tt, t0, ts in TOK:
                    pt = p_sc.tile([128, 1024], F32, tag="ps",
                                   name=f"sc_{b}_{h}_{tt}")
                    nc.tensor.matmul(pt[0:ts, 0:512], kh_t[:, t0:t0 + ts],
                                     qh_t[:, 0:512], start=True, stop=True)
                    nc.tensor.matmul(pt[0:ts, 512:578], kh_t[:, t0:t0 + ts],
                                     qh_t[:, 512:578], start=True, stop=True)
                    et = p_expT.tile([128, SP], BF16, tag="expT",
                                     name=f"et_{b}_{h}_{tt}")
                    nc.scalar.activation(et[0:ts, 0:SP], pt[0:ts, 0:SP],
                                         AF.Exp, scale=SCALE)
                    expTs.append(et)
                    filler.take(fill_exp)
                pv = p_pv.tile([128, 1024], F32, tag="ps", name=f"pv_{b}_{h}")
                for tt, t0, ts in TOK:
                    et = expTs[tt]
                    vsl = vsb[b][tt][0:ts, h * VG:(h + 1) * VG]
                    nc.tensor.matmul(pv[0:VG, 0:512], vsl, et[0:ts, 0:512],
                                     start=(tt == 0), stop=(tt == TT - 1))
                    nc.tensor.matmul(pv[0:VG, 512:578], vsl, et[0:ts, 512:578],
                                     start=(tt == 0), stop=(tt == TT - 1))
                    filler.take(fill_pv)
                # one copy brings values AND the denominator row to SBUF
                pvs = p_pvs.tile([VG, SP], F32, tag="pvs", name=f"pvs_{b}_{h}")
                nc.vector.tensor_copy(pvs[:, 0:SP], pv[0:VG, 0:SP])
                # custom-DVE op needs base partition 0: run it over all VG
                # rows (same per-lane cost); only row 96 (the denominator)
                # is meaningful.
                recf = p_rec.tile([VG, SP], F32, tag="recf", name=f"recf_{b}_{h}")
                nc.vector.reciprocal_approx_fast(recf[:, 0:SP], pvs[:, 0:SP])
                rec = p_rec.tile([1, SP], BF16, tag="rec", name=f"rec_{b}_{h}")
                nc.vector.tensor_copy(rec[:, 0:SP], recf[DEN:DEN + 1, 0:SP])
                if pend is not None:
                    finish_head(b, *pend)
                pend = (h, rec, pvs)
            finish_head(b, *pend)

        # ================= phase drivers =================

        def drain(gen):
            for _ in gen:
                pass

        # ---- P1: v-proj(b0,b1) + q|k-proj(b0), coarse interleave ----
        c1b0 = [gen_c1_m(0, m) for m in range(MT)]
        p1 = []
        bpieces = [(b, ci) for b in range(B) for ci in range(3)]
        mi = 0
        for i, (b, ci) in enumerate(bpieces):
            p1.append(gen_b_chunk(b, ci))
            take = 4 if i < 4 else 3
            for _ in range(take):
                if mi < MT:
                    p1.append(c1b0[mi])
                    mi += 1
        while mi < MT:
            p1.append(c1b0[mi])
            mi += 1
        primed = 0
        for i in range(len(p1)):
            while primed < min(i + 3, len(p1)):
                try:
                    next(p1[primed])
                except StopIteration:
                    pass
                primed += 1
            drain(p1[i])

        # ---- P2: attention(b0) with q|k-proj(b1) as filler ----
        f2 = Filler()
        c1b1 = [gen_c1_m(1, m) for m in range(MT)]
        gstate = {"j": 0}

        def gate2(h):
            jm = min((88 * (h + 2)) // 128, KT - 1)
            while gstate["j"] <= jm:
                j = gstate["j"]
                f2.add(c1b1[j])          # q m-tile j
                f2.add(c1b1[KT + j])     # k m-tile j
                emit_wp_load()
                gstate["j"] += 1

        attention_phase(0, f2, fill_exp=1100, fill_pv=650, gate=gate2)
        f2.drain()
        while len(wp) < KT:
            emit_wp_load()

        # ---- P3: attention(b1) with out-proj(b0) as filler ----
        f3 = Filler()
        for tt, t0, ts in TOK:
            for cgi in range(2):
                f3.add(gen_d_piece(0, tt, t0, ts, cgi, p_big))
        attention_phase(1, f3, fill_exp=650, fill_pv=390)
        f3.drain()

        # ---- P4: out-proj(b1), psum rotating across all three pools ----
        pools4 = [p_big, p_sc, p_pv]
        pi = 0
        d4 = []
        for tt, t0, ts in TOK:
            for cgi in range(2):
                d4.append(gen_d_piece(1, tt, t0, ts, cgi, pools4[pi % 3]))
                pi += 1
        primed = 0
        for i in range(len(d4)):
            while primed < min(i + 2, len(d4)):
                try:
                    next(d4[primed])
                except StopIteration:
                    pass
                primed += 1
            drain(d4[i])

    nc.compile()
    return nc


_NC_CACHE = None


def _get_nc():
    global _NC_CACHE
    if _NC_CACHE is None:
        _NC_CACHE = build_program()
    return _NC_CACHE


def make_in_maps(hidden_states, w_qkv, b_qkv, w_proj, b_proj):
    bf16 = ml_dtypes.bfloat16
    hs = np.asarray(hidden_states, dtype=np.float32)
    w_qkv = np.ascontiguousarray(np.asarray(w_qkv, dtype=np.float32))
    b_qkv = np.asarray(b_qkv, dtype=np.float32)
    w_proj = np.ascontiguousarray(np.asarray(w_proj, dtype=np.float32))
    b_proj = np.asarray(b_proj, dtype=np.float32)

    # q|k weights -> m-stripe layout: wqk_m[p, m*1408 + k*128 + c]
    #   = w_qkv[k*128 + p, m*128 + c]
    wqk = w_qkv[:, :2 * D].reshape(KT, 128, MT, 128)
    wqk_m = np.ascontiguousarray(
        wqk.transpose(1, 2, 0, 3).reshape(128, MT * KT * 128)).astype(bf16)
    # v / proj weights -> k-stripe layout: w_r[p, k*1408 + c] = w[k*128+p, c]
    wv_r = np.ascontiguousarray(
        w_qkv[:, 2 * D:].reshape(KT, 128, D).transpose(1, 0, 2)
        .reshape(128, KT * D)).astype(bf16)
    wp_r = np.ascontiguousarray(
        w_proj.reshape(KT, 128, D).transpose(1, 0, 2)
        .reshape(128, KT * D)).astype(bf16)

    bqk_col = np.ascontiguousarray(b_qkv[:2 * D].reshape(2 * D, 1))
    bv_row = np.ascontiguousarray(b_qkv[2 * D:].reshape(1, D)).astype(bf16)
    bp_row = np.ascontiguousarray(b_proj.reshape(1, D)).astype(bf16)
    ones_f = np.ones((1, 128), np.float32)
    ones_bf = np.ones((1, 128), bf16)

    in_maps = []
    for c in range(N_CORES):
        xt = np.ones((D, B * SP), bf16)
        for b in range(B):
            xs = hs[c * B + b]                       # [S, D]
            xt[:, b * SP:b * SP + S] = xs.T.astype(bf16)
        in_maps.append({
            "xT_bf": xt,
            "wqk_m": wqk_m,
            "wv_r": wv_r,
            "wp_r": wp_r,
            "b_qk_col": bqk_col,
            "b_v_row": bv_row,
            "b_p_row": bp_row,
            "ones_f": ones_f,
            "ones_bf": ones_bf,
        })
    return in_maps


def kernel(hidden_states, w_qkv, b_qkv, w_proj, b_proj):
    nc = _get_nc()
    in_maps = make_in_maps(hidden_states, w_qkv, b_qkv, w_proj, b_proj)
    res = run_bass_kernel_spmd(nc, in_maps, list(range(N_CORES)))
    out = np.concatenate(
        [res.results[c]["out"].reshape(B, S, D) for c in range(N_CORES)], axis=0)
    return out.astype(np.float32)


if __name__ == "__main__":
    rng = np.random.default_rng(0)
    hs = rng.standard_normal((B_TOTAL, S, D), dtype=np.float32)
    wq = rng.standard_normal((D, 3 * D), dtype=np.float32) * D ** -0.5
    bq = rng.standard_normal(3 * D).astype(np.float32) * 0.02
    wp = rng.standard_normal((D, D), dtype=np.float32) * D ** -0.5
    bp = rng.standard_normal(D).astype(np.float32) * 0.02
    o = kernel(hidden_states=hs, w_qkv=wq, b_qkv=bq, w_proj=wp, b_proj=bp)
    print(o.shape, o.dtype)
